# revision 24
# baseline (speedup 1.0000x reference)
"""BiMamba Trainium2 kernel.

Sharding: 8 cores = (direction f/r) x (batch 2) x (d_inner half 2), SPMD
(one program, per-core data).  The host permutes channel order so each
core's own 512 scan channels occupy positions 0..511; xi/conv are computed
for all 1024 channels on every core (x_proj needs the full d_inner
contraction); z/dt/scan/out_proj cover only the own half.  Partial
out_proj results are summed on the host; the reverse direction is flipped
on the host.

Device pipeline (feature-major [feature, token] layouts):
  A) in_proj -> xi (PE, f32r); xi drained PSUM->SBUF bf16 on ScalarE;
     depthwise conv as a scalar_tensor_tensor multiply-add chain on
     VectorE/GpSimd (keeps PE free); silu via the native Silu table on
     ScalarE; x_proj accumulated over all 8 channel tiles (bf16 matmuls);
     z -> native silu -> gT
  B) x_proj psum -> dt_raw (f32r) and B/C rows (bf16) via ScalarE copies;
     dt_proj -> native Softplus -> dtT (bf16); u = dt*xc (bf16 2x)
  C) selective scan, per (pt pair, state s): broadcast B_s/C_s rows to 128
     partitions via partition-step-0 DMA on the SP queue (bf16);
     dA = exp(A_s*dt) on ScalarE (bf16), dBu/hc multiplies split between
     VectorE and GpSimd (bf16 2x on DVE), full-length tensor_tensor_scan
     (fp32 state), y = D*xc + sum_s hc via identity/diag matmuls into PSUM
  D) y_gated = y_psum * silu(z) -> f32r; out_proj partial -> DRAM from PSUM

Env knobs: BIMAMBA_LOOP (hw timing loop), BIMAMBA_SIMSAFE (avoid
Silu/Softplus tables so CoreSim's executor can run), BIMAMBA_SCANENG
(dve|pool), BIMAMBA_MULPOOL (how many of the 4 per-(pair,s) multiplies go
to GpSimd), BIMAMBA_CONVPOOL (conv chain tiles on GpSimd: bitmask-ish
count 0..8).
"""
import os
from contextlib import ExitStack

import numpy as np

import concourse.bacc as bacc
import concourse.bass as bass
import concourse.tile as tile
from concourse import mybir
from concourse.bass_utils import run_bass_kernel_spmd

F32 = mybir.dt.float32
BF16 = mybir.dt.bfloat16
F32R = mybir.dt.float32r
AF = mybir.ActivationFunctionType
OP = mybir.AluOpType
NPBF16 = mybir.dt.np(mybir.dt.bfloat16)

DIM = 512
D_STATE = 16
D_CONV = 4
D_INNER = 1024
DT_RANK = 32
B_SZ = 2
SEQ = 2048
HALF = 512
NPT = HALF // 128     # 4 own-channel partition tiles
NFT = D_INNER // 128  # 8 full-channel partition tiles
NC_ = SEQ // 512      # 4 token chunks
NXD = DT_RANK + 2 * D_STATE  # 64

_PROG_CACHE = {}


def _build_program():
    key = "nc"
    if key in _PROG_CACHE:
        return _PROG_CACHE[key]
    simsafe = bool(os.environ.get("BIMAMBA_SIMSAFE"))
    scaneng = os.environ.get("BIMAMBA_SCANENG", "dve")
    mulpool = int(os.environ.get("BIMAMBA_MULPOOL", "3"))
    # conv engine split over the 8 channel tiles: pe,dve,pool counts
    conv_cfg = os.environ.get("BIMAMBA_CONV", "mix:4,4,0")
    if conv_cfg.startswith("mix:"):
        cpe, cdve, cpool = (int(x) for x in conv_cfg[4:].split(","))
    else:
        cpe, cdve, cpool = {"pe": (8, 0, 0), "dve": (0, 8, 0),
                            "pool": (0, 0, 8)}[conv_cfg]
    assert cpe + cdve + cpool == 8
    rem = {"pe": cpe, "dve": cdve, "pool": cpool}
    conv_assign = []
    i = 0
    while len(conv_assign) < 8:
        e = ("pe", "dve", "pool")[i % 3]
        if rem[e] > 0:
            conv_assign.append(e)
            rem[e] -= 1
        i += 1

    nc = bacc.Bacc("TRN2", target_bir_lowering=False, debug=False)

    xT = nc.dram_tensor("xT", [128, 4, SEQ], F32R, kind="ExternalInput")
    w_in = nc.dram_tensor("w_in", [128, 4, D_INNER + HALF], F32R, kind="ExternalInput")
    convw = nc.dram_tensor("convw", [128, NFT, D_CONV], F32, kind="ExternalInput")
    convd = nc.dram_tensor("convd", [128, NFT, D_CONV, 128], BF16, kind="ExternalInput")
    convb = nc.dram_tensor("convb", [128, NFT, 1], F32, kind="ExternalInput")
    w_xp = nc.dram_tensor("w_xp", [128, NFT, NXD], BF16, kind="ExternalInput")
    w_dt = nc.dram_tensor("w_dt", [DT_RANK, HALF], F32R, kind="ExternalInput")
    dtb = nc.dram_tensor("dtb", [128, NPT, 1], F32, kind="ExternalInput")
    Acol = nc.dram_tensor("Acol", [128, NPT, D_STATE], F32, kind="ExternalInput")
    diagD = nc.dram_tensor("diagD", [128, NPT, 128], BF16, kind="ExternalInput")
    ident = nc.dram_tensor("ident", [128, 128], BF16, kind="ExternalInput")
    w_out = nc.dram_tensor("w_out", [128, NPT, DIM], BF16, kind="ExternalInput")
    oT = nc.dram_tensor("oT", [128, 4, SEQ], F32, kind="ExternalOutput")

    loop_n = int(os.environ.get("BIMAMBA_LOOP", "0"))
    with tile.TileContext(nc) as tc, ExitStack() as est:
        if loop_n > 1:
            est.enter_context(tc.For_i(0, loop_n, 1))
        pP = est.enter_context(tc.tile_pool(name="pP", bufs=1))
        pDram = est.enter_context(tc.tile_pool(name="pDram", bufs=1, space="DRAM"))
        bcd = pDram.tile([2 * D_STATE, SEQ], BF16)

        gT = pP.tile([128, NPT, SEQ], BF16)       # silu(z), own half
        xc_own = pP.tile([128, NPT, SEQ], BF16)   # silu(conv(xi)), own half
        dbc_raw = pP.tile([DT_RANK, SEQ], F32R)   # dt_raw rows
        bcb = pP.tile([2 * D_STATE, SEQ], BF16)   # rows 0..15 = B, 16..31 = C

        with tc.tile_pool(name="psX", bufs=4, space="PSUM") as psX:
            psx = []
            for _c in range(NC_):
                psx_t = psX.tile([NXD, 512], F32, tag="xp")
                psx.append(psx_t)

            # ---------- Phase A ----------
            with tc.tile_pool(name="pA", bufs=1) as pA, \
                 tc.tile_pool(name="pAw", bufs=2) as pAw, \
                 tc.tile_pool(name="pXi", bufs=2) as pXi, \
                 tc.tile_pool(name="pCv", bufs=3) as pCv, \
                 tc.tile_pool(name="psA", bufs=3, space="PSUM") as psA:
                sb_xT = pA.tile([128, 4, SEQ], F32R)
                for c in range(NC_):
                    nc.sync.dma_start(sb_xT[:, :, c * 512:(c + 1) * 512],
                                      xT[:, :, c * 512:(c + 1) * 512])
                sb_cb = pA.tile([128, NFT, 1], F32)
                sb_wxp = pA.tile([128, NFT, NXD], BF16)
                sb_cw = pA.tile([128, NFT, D_CONV], F32)
                sb_cd = pA.tile([128, NFT, D_CONV, 128], BF16)
                # small tables go on the gpsimd queue so they don't delay
                # the xT chunks on the SP queue
                nc.gpsimd.dma_start(sb_cb[:], convb[:])
                nc.gpsimd.dma_start(sb_wxp[:], w_xp[:])
                nc.gpsimd.dma_start(sb_cw[:], convw[:])
                nc.gpsimd.dma_start(sb_cd[:], convd[:])

                # xi/conv channel tiles first (x_proj finishes earlier so the
                # scan phase can start); z tiles last
                for m in list(range(8)) + list(range(8, 12)):
                    win_m = pAw.tile([128, 4, 128], F32R, tag="win")
                    # weight loads on the scalar queue (idle early)
                    nc.scalar.dma_start(win_m[:], w_in[:, :, m * 128:(m + 1) * 128])
                    xi_pad = None
                    if m < 8:
                        xi_pad = pXi.tile([128, 3 + SEQ], BF16, tag="xi_pad")
                        nc.vector.memset(xi_pad[:, 0:3], 0.0)
                        cmode = conv_assign[m]
                    for c in range(NC_):
                        ps = psA.tile([128, 512], F32, tag="mm", bufs=2)
                        for k in range(4):
                            nc.tensor.matmul(
                                ps[:], win_m[:, k, :],
                                sb_xT[:, k, c * 512:(c + 1) * 512],
                                start=(k == 0), stop=(k == 3))
                        if m < 8:
                            # drain xi PSUM -> SBUF (bf16) on ScalarE
                            nc.scalar.activation(
                                xi_pad[:, 3 + c * 512: 3 + (c + 1) * 512], ps[:],
                                AF.Copy)
                            # depthwise conv, engine per tile assignment
                            if cmode == "pe":
                                pacc = psA.tile([128, 512], F32, tag="cmm", bufs=2)
                                for k in range(D_CONV):
                                    nc.tensor.matmul(
                                        pacc[:], sb_cd[:, m, k, :],
                                        xi_pad[:, c * 512 + k: c * 512 + k + 512],
                                        start=(k == 0), stop=(k == D_CONV - 1))
                            elif cmode == "dve":
                                pacc = pCv.tile([128, 512], F32, tag="cv")
                                nc.vector.tensor_scalar_mul(
                                    pacc[:], xi_pad[:, c * 512: c * 512 + 512],
                                    sb_cw[:, m, 0:1])
                                for k in range(1, D_CONV):
                                    pnew = pCv.tile([128, 512], F32, tag="cv")
                                    nc.vector.scalar_tensor_tensor(
                                        pnew[:],
                                        xi_pad[:, c * 512 + k: c * 512 + k + 512],
                                        sb_cw[:, m, k:k + 1], pacc[:],
                                        OP.mult, OP.add)
                                    pacc = pnew
                            else:  # pool: tensor_scalar_mul + tensor_add only
                                parts = []
                                for k in range(D_CONV):
                                    pk = pCv.tile([128, 512], F32, tag="cvp", bufs=8)
                                    nc.gpsimd.tensor_scalar_mul(
                                        pk[:],
                                        xi_pad[:, c * 512 + k: c * 512 + k + 512],
                                        sb_cw[:, m, k:k + 1])
                                    parts.append(pk)
                                s01 = pCv.tile([128, 512], F32, tag="cvp", bufs=8)
                                nc.gpsimd.tensor_add(s01[:], parts[0][:], parts[1][:])
                                s23 = pCv.tile([128, 512], F32, tag="cvp", bufs=8)
                                nc.gpsimd.tensor_add(s23[:], parts[2][:], parts[3][:])
                                pacc = pCv.tile([128, 512], F32, tag="cvp", bufs=8)
                                nc.gpsimd.tensor_add(pacc[:], s01[:], s23[:])
                            if m < NPT:
                                xco = xc_own[:, m, c * 512:(c + 1) * 512]
                            else:
                                xco_t = pXi.tile([128, 512], BF16, tag="xco")
                                xco = xco_t[:]
                            # silu(v), v = conv + bias
                            if simsafe:
                                sg = pCv.tile([128, 512], F32, tag="sg")
                                nc.scalar.activation(sg[:], pacc[:], AF.Sigmoid,
                                                     bias=sb_cb[:, m, :])
                                nc.vector.scalar_tensor_tensor(
                                    xco, pacc[:], sb_cb[:, m, :], sg[:],
                                    OP.add, OP.mult)
                            else:
                                nc.scalar.activation(xco, pacc[:], AF.Silu,
                                                     bias=sb_cb[:, m, :])
                            # accumulate x_proj contribution of this tile
                            nc.tensor.matmul(
                                psx[c][:], sb_wxp[:, m, :], xco,
                                start=(m == 0), stop=(m == 7))
                        else:
                            # silu(z) on ScalarE only
                            if simsafe:
                                sgz = pXi.tile([128, 512], F32, tag="sgz")
                                nc.scalar.activation(sgz[:], ps[:], AF.Sigmoid)
                                nc.vector.tensor_mul(
                                    gT[:, m - 8, c * 512:(c + 1) * 512],
                                    ps[:], sgz[:])
                            else:
                                nc.scalar.activation(
                                    gT[:, m - 8, c * 512:(c + 1) * 512],
                                    ps[:], AF.Silu)

            # unload x_proj accumulators (still inside psX scope) on DVE,
            # which is idle at the A->B transition while ScalarE works
            for c in range(NC_):
                nc.vector.tensor_copy(dbc_raw[:, c * 512:(c + 1) * 512],
                                      psx[c][0:DT_RANK, :])
                nc.vector.tensor_copy(bcb[:, c * 512:(c + 1) * 512],
                                      psx[c][DT_RANK:NXD, :])
        # stage B/C rows in DRAM so the per-s broadcast DMA can use a
        # partition-step-0 source (SBUF sources reject it)
        nc.sync.dma_start(bcd[:], bcb[:])

        # ---------- Phase B ----------
        pBCD = est.enter_context(tc.tile_pool(name="pBCD", bufs=1))
        dtT = pBCD.tile([128, NPT, SEQ], BF16)
        uT = pBCD.tile([128, NPT, SEQ], BF16)
        sb_A = pBCD.tile([128, NPT, D_STATE], F32)
        sb_dD = pBCD.tile([128, NPT, 128], BF16)
        sb_id = pBCD.tile([128, 128], BF16)
        y_g = pBCD.tile([128, NPT, SEQ], BF16)
        nc.sync.dma_start(sb_A[:], Acol[:])
        nc.sync.dma_start(sb_dD[:], diagD[:])
        nc.sync.dma_start(sb_id[:], ident[:])

        with tc.tile_pool(name="pB", bufs=1) as pB, \
             tc.tile_pool(name="psB", bufs=2, space="PSUM") as psB:
            sb_wdt = pB.tile([DT_RANK, HALF], F32R)
            sb_dtb = pB.tile([128, NPT, 1], F32)
            nc.sync.dma_start(sb_wdt[:], w_dt[:])
            nc.sync.dma_start(sb_dtb[:], dtb[:])
            # softplus(w) = ln(1 + exp(w)); w = psum + dt_bias.  Processed
            # per channel tile (Exp x4 then Ln then u-mult) so the first
            # scan states only wait on the first two tiles.
            spe = pB.tile([128, NPT, SEQ], F32)
            for mt in range(NPT):
                for c in range(NC_):
                    ps3 = psB.tile([128, 512], F32, tag="mm")
                    nc.tensor.matmul(
                        ps3[:], sb_wdt[:, mt * 128:(mt + 1) * 128],
                        dbc_raw[:, c * 512:(c + 1) * 512], start=True, stop=True)
                    nc.scalar.activation(
                        spe[:, mt, c * 512:(c + 1) * 512], ps3[:], AF.Exp,
                        bias=sb_dtb[:, mt, :])
                nc.scalar.activation(dtT[:, mt, :], spe[:, mt, :], AF.Ln,
                                     bias=1.0)
                nc.vector.tensor_mul(uT[:, mt, :], dtT[:, mt, :],
                                     xc_own[:, mt, :])

        # ---------- Phase C: selective scan ----------
        # Software-pipelined per state s: broadcasts prefetched 2 ahead,
        # dA/dBu 1 ahead, hc + identity-matmul accumulation deferred 1
        # behind, so neither DVE nor GpSimd ever stalls on an intra-s dep.
        if scaneng == "pool":
            seng, meng = nc.gpsimd, nc.vector
            hc_engs = (nc.vector, nc.vector)
        else:
            seng, meng = nc.vector, nc.gpsimd
            hc_engs = (nc.gpsimd, nc.vector)
        with tc.tile_pool(name="pC", bufs=3) as pC, \
             tc.tile_pool(name="pCb", bufs=4) as pCb, \
             tc.tile_pool(name="psC", bufs=8, space="PSUM") as psC:
            finalize_prev = None
            for pair in range(2):
                pts = (2 * pair, 2 * pair + 1)

                def load_bc(s):
                    B_bc = pCb.tile([128, SEQ], BF16, tag="bbc", bufs=4)
                    C_bc = pCb.tile([128, SEQ], BF16, tag="cbc", bufs=5)
                    brow = bcd[s:s + 1, :]
                    crow = bcd[D_STATE + s:D_STATE + s + 1, :]
                    nc.sync.dma_start(B_bc[:], bass.AP(
                        tensor=brow.tensor, offset=brow.offset,
                        ap=[[0, 128]] + list(brow.ap[1:])))
                    nc.sync.dma_start(C_bc[:], bass.AP(
                        tensor=crow.tensor, offset=crow.offset,
                        ap=[[0, 128]] + list(crow.ap[1:])))
                    return B_bc, C_bc

                def make_dA_dBu(s, bc):
                    out = []
                    for ptl, pt in enumerate(pts):
                        dA = pC.tile([128, SEQ], BF16, tag="dA", bufs=5)
                        nc.scalar.activation(dA[:], dtT[:, pt, :], AF.Exp,
                                             scale=sb_A[:, pt, s:s + 1])
                        dBu = pC.tile([128, SEQ], BF16, tag="dBu", bufs=5)
                        meng.tensor_mul(dBu[:], uT[:, pt, :], bc[0][:])
                        out.append((dA, dBu))
                    return out

                # prologue: prefetch broadcasts and first dA/dBu before the
                # previous pair's drain+gate lump so DVE/GpSimd never idle
                bc = {0: load_bc(0)}
                bc[1] = load_bc(1)
                ab = {0: make_dA_dBu(0, bc[0])}

                if finalize_prev is not None:
                    finalize_prev()
                    finalize_prev = None

                # y accumulators: one PSUM bank per (pt-in-pair, token chunk)
                yps = {}
                for ptl, pt in enumerate(pts):
                    for q in range(NC_):
                        yps_t = psC.tile([128, 512], F32, tag="yps")
                        yps[(ptl, q)] = yps_t
                        # initialize with D * xc via diag matmul
                        nc.tensor.matmul(
                            yps_t[:], sb_dD[:, pt, :],
                            xc_own[:, pt, q * 512:(q + 1) * 512],
                            start=True, stop=False, skip_group_check=True)

                def emit_hc(s, hs, C_bc, yps=yps, pts=pts):
                    for ptl, pt in enumerate(pts):
                        hc = pC.tile([128, SEQ], BF16, tag="hc", bufs=2)
                        # hc0 on GpSimd; hc1 on DVE (engine balance)
                        heng = nc.gpsimd if ptl == 0 else nc.vector
                        heng.tensor_mul(hc[:], hs[ptl][:], C_bc[:])
                        for q in range(NC_):
                            nc.tensor.matmul(
                                yps[(ptl, q)][:], sb_id[:],
                                hc[:, q * 512:(q + 1) * 512],
                                start=False, stop=(s == D_STATE - 1),
                                skip_group_check=True)

                hprev = None
                for s in range(D_STATE):
                    hs = []
                    for ptl, pt in enumerate(pts):
                        dA, dBu = ab[s][ptl]
                        h = pC.tile([128, SEQ], BF16, tag="h", bufs=4)
                        seng.tensor_tensor_scan(h[:], dA[:], dBu[:], 0.0,
                                                OP.mult, OP.add)
                        hs.append(h)
                    if s + 2 < D_STATE:
                        bc[s + 2] = load_bc(s + 2)
                    if s + 1 < D_STATE:
                        ab[s + 1] = make_dA_dBu(s + 1, bc[s + 1])
                    if hprev is not None:
                        emit_hc(s - 1, hprev, bc[s - 1][1])
                        del bc[s - 1], ab[s - 1]
                    hprev = hs
                emit_hc(D_STATE - 1, hprev, bc[D_STATE - 1][1])

                def finalize(yps=yps, pts=pts):
                    # drain y PSUM -> SBUF bf16 on ScalarE, then gate
                    # y_g = y * silu(z) split across DVE/GpSimd (both read
                    # SBUF only, freeing DVE at the pair boundary)
                    for ptl, pt in enumerate(pts):
                        for q in range(NC_):
                            ysb = pC.tile([128, 512], BF16, tag="ysb", bufs=4)
                            nc.scalar.activation(ysb[:], yps[(ptl, q)][:],
                                                 AF.Copy)
                            geng = nc.vector if ptl else nc.gpsimd
                            geng.tensor_mul(
                                y_g[:, pt, q * 512:(q + 1) * 512],
                                ysb[:],
                                gT[:, pt, q * 512:(q + 1) * 512])
                finalize_prev = finalize
            finalize_prev()

        # ---------- Phase D: out_proj ----------
        with tc.tile_pool(name="pD", bufs=1) as pD, \
             tc.tile_pool(name="pDo", bufs=3) as pDo, \
             tc.tile_pool(name="psD", bufs=3, space="PSUM") as psD:
            sb_wout = pD.tile([128, NPT, DIM], BF16)
            nc.sync.dma_start(sb_wout[:], w_out[:])
            for mt in range(NPT):
                for c in range(NC_):
                    ps4 = psD.tile([128, 512], F32, tag="mm")
                    for k in range(NPT):
                        nc.tensor.matmul(
                            ps4[:], sb_wout[:, k, mt * 128:(mt + 1) * 128],
                            y_g[:, k, c * 512:(c + 1) * 512],
                            start=(k == 0), stop=(k == NPT - 1))
                    ot = pDo.tile([128, 512], F32, tag="ot")
                    nc.scalar.activation(ot[:], ps4[:], AF.Copy)
                    nc.sync.dma_start(oT[:, mt, c * 512:(c + 1) * 512], ot[:])

    nc.compile()
    _PROG_CACHE[key] = nc
    return nc


def _prep_core_inputs(x, params, direction, batch, half):
    in_w, conv_w, conv_b, xproj_w, dt_w, dt_b, A_log, D, out_w = params
    xb = x[batch]
    if direction == 1:
        xb = xb[::-1]
    xT = np.ascontiguousarray(xb.T)

    own = np.arange(half * HALF, (half + 1) * HALF)
    other = np.arange((1 - half) * HALF, (2 - half) * HALF)
    perm = np.concatenate([own, other])

    w_in = np.concatenate([in_w[perm], in_w[D_INNER + own]], axis=0).T  # [512, 1536]
    cw = conv_w[perm, 0, :]                                            # [1024, 4]
    convw_ = cw.reshape(NFT, 128, D_CONV).transpose(1, 0, 2)           # [128, NFT, 4]
    ii = np.arange(128)
    convb_ = conv_b[perm].reshape(NFT, 128, 1).transpose(1, 0, 2)
    w_xp = xproj_w[:, perm].T.reshape(NFT, 128, -1).transpose(1, 0, 2)
    w_dt_ = np.ascontiguousarray(dt_w[own].T)
    dtb_ = dt_b[own].reshape(NPT, 128, 1).transpose(1, 0, 2)
    Acol_ = (-np.exp(A_log[own])).reshape(NPT, 128, D_STATE).transpose(1, 0, 2)
    cd = np.zeros((128, NFT, D_CONV, 128), np.float32)
    for ft in range(NFT):
        for k in range(D_CONV):
            cd[ii, ft, k, ii] = convw_[ii, ft, k]
    dD = np.zeros((128, NPT, 128), np.float32)
    Dr = D[own].reshape(NPT, 128)
    for ptn in range(NPT):
        dD[ii, ptn, ii] = Dr[ptn]
    w_out_ = out_w[:, own].T.reshape(NPT, 128, DIM).transpose(1, 0, 2)

    def c32(a):
        return np.ascontiguousarray(a, dtype=np.float32)

    def cbf(a):
        return np.ascontiguousarray(a, dtype=NPBF16)

    return {
        "xT": c32(xT.reshape(4, 128, SEQ).transpose(1, 0, 2)),
        "w_in": c32(w_in.reshape(4, 128, -1).transpose(1, 0, 2)),
        "convw": c32(convw_),
        "convd": cbf(cd),
        "convb": c32(convb_),
        "w_xp": cbf(w_xp),
        "w_dt": c32(w_dt_),
        "dtb": c32(dtb_),
        "Acol": c32(Acol_),
        "diagD": cbf(dD),
        "ident": np.eye(128, dtype=NPBF16),
        "w_out": cbf(w_out_),
    }


def _run(nc, in_maps):
    if os.environ.get("BIMAMBA_SIM"):
        from concourse.bass_interp import CoreSim
        results = []
        n = int(os.environ.get("BIMAMBA_SIM_CORES", "8"))
        for m in in_maps[:n]:
            sim = CoreSim(nc)
            for k, v in m.items():
                sim.tensor(k)[:] = v
            sim.simulate()
            results.append({"oT": np.array(sim.tensor("oT"))})
        return results
    return run_bass_kernel_spmd(nc, in_maps, core_ids=list(range(8))).results


def _prep_all(inputs):
    x = np.asarray(inputs["x"], np.float32)
    names = ["in_w", "conv_w", "conv_b", "xproj_w", "dt_w", "dt_b", "A_log", "D", "out_w"]
    fp = tuple(np.asarray(inputs["f_" + n], np.float32) for n in names)
    rp = tuple(np.asarray(inputs["r_" + n], np.float32) for n in names)
    in_maps = []
    meta = []
    for d in (0, 1):
        for b in range(B_SZ):
            for h in (0, 1):
                in_maps.append(_prep_core_inputs(x, fp if d == 0 else rp, d, b, h))
                meta.append((d, b, h))
    return in_maps, meta


def kernel(**inputs):
    nc = _build_program()
    in_maps, meta = _prep_all(inputs)
    results = _run(nc, in_maps)

    acc = np.zeros((2, B_SZ, SEQ, DIM), np.float32)
    for (d, b, h), r in zip(meta, results):
        oTv = r["oT"]
        o = oTv.transpose(1, 0, 2).reshape(DIM, SEQ).T
        if d == 1:
            o = o[::-1]
        acc[d, b] += o
    out = 0.5 * (acc[0] + acc[1])
    return out.astype(np.float32)
